# revision 1
# baseline (speedup 1.0000x reference)
"""GAT 2-layer + BN/ELU + linear head on 8 Trainium2 cores.

Strategy (per spec sharding_hint): nodes partitioned across cores by
destination (graph parallel), degree-sorted tiles of 128 dsts for uniform
per-tile edge batches; edges routed to the dst owner. Layer-1 src features
(x rows, host-known input) are routed with the edges (halo exchange done
during input sharding). Layer-2 src features (h1, device-computed) are
gathered on-device from a replicated node table built via AllGather.

Aggregation layout ("identity-dst"): a tile = 128 dst nodes on the 128
partitions; batch b holds each dst's b-th incoming edge at its partition.
Per-edge softmax scalars then live at [dst-partition, batch] and the
weighted sum over incoming edges is a free-dim reduce.
"""
import sys
sys.path.insert(0, "/opt/trn_rl_repo")
from contextlib import ExitStack

import numpy as np
import ml_dtypes

import concourse.mybir as mybir
import concourse.tile as tile
from concourse import bacc
from concourse.bass import IndirectOffsetOnAxis
from concourse.bass_utils import run_bass_kernel_spmd

P = 128
f32 = mybir.dt.float32
bf16 = mybir.dt.bfloat16
i32 = mybir.dt.int32

NEG = -1e30  # esrc sentinel: exp(leaky(NEG + edst)) == 0

# full-size problem config
CFG_FULL = dict(N=100000, D=128, NC=40, NCORES=8, TPC=98)  # tiles per core


def leaky_exp(nc, sb, esrc_ap, edst_col, nb, denom):
    """p = exp(leaky_relu(esrc + edst, 0.2)); denom = row-sum(p). [P, nb]."""
    s = sb.tile([P, nb], f32, tag="s")
    s2 = sb.tile([P, nb], f32, tag="s2")
    p = sb.tile([P, nb], f32, tag="p")
    nc.vector.tensor_scalar_add(s[:], esrc_ap, edst_col)
    nc.vector.tensor_scalar_mul(s2[:], s[:], 0.2)
    nc.vector.tensor_tensor(out=s[:], in0=s[:], in1=s2[:],
                            op=mybir.AluOpType.max)
    nc.scalar.activation(p[:], s[:], mybir.ActivationFunctionType.Exp,
                         accum_out=denom[:])
    return p


def aggregate(nc, sb, ps, G3, p, denom, nb, ident, Wb, z_psum):
    """agg = sum_b (p_norm[:,b] * G3[:,b,:]);  z_psum = (aggT)^T @ Wb."""
    inv = sb.tile([P, 1], f32, tag="inv")
    nc.vector.reciprocal(inv[:], denom[:])
    pn = sb.tile([P, nb], f32, tag="pn")
    nc.vector.tensor_scalar_mul(pn[:], p[:], inv[:])
    Gw = sb.tile([P, nb * 128], bf16, tag="Gw")
    nc.vector.tensor_tensor(
        out=Gw[:].rearrange("p (b f) -> p b f", f=128),
        in0=G3, in1=pn[:].unsqueeze(2).to_broadcast([P, nb, 128]),
        op=mybir.AluOpType.mult)
    agg = sb.tile([P, 128], f32, tag="agg")
    nc.vector.tensor_reduce(
        agg[:], Gw[:].rearrange("p (b f) -> p b f", f=128).transpose([0, 2, 1]),
        axis=mybir.AxisListType.X, op=mybir.AluOpType.add)
    aggT_ps = ps.tile([P, P], f32, tag="aggT_ps", space="PSUM")
    nc.tensor.transpose(out=aggT_ps[:], in_=agg[:], identity=ident[:])
    aggT = sb.tile([P, P], bf16, tag="aggT")
    nc.vector.tensor_copy(aggT[:], aggT_ps[:])
    nc.tensor.matmul(z_psum[:], lhsT=aggT[:], rhs=Wb[:], start=True, stop=True)


def stats_accum(nc, sb, z_sb, ones, st_z, st_z2, j, last):
    """Accumulate column sums of z and z^2 into persistent PSUM rows."""
    zb = sb.tile([P, 128], bf16, tag="zb")
    nc.vector.tensor_copy(zb[:], z_sb)
    z2 = sb.tile([P, 128], bf16, tag="z2")
    nc.vector.tensor_tensor(out=z2[:], in0=z_sb, in1=z_sb,
                            op=mybir.AluOpType.mult)
    nc.tensor.matmul(st_z[:], lhsT=ones[:], rhs=zb[:],
                     start=(j == 0), stop=last)
    nc.tensor.matmul(st_z2[:], lhsT=ones[:], rhs=z2[:],
                     start=(j == 0), stop=last)


def bn_consts(nc, sb, st_sb, gamma, beta, n_nodes, A, B):
    """A = gamma/sqrt(var+eps), B = beta - mu*A  from [1,256] global sums."""
    mu = sb.tile([1, 128], f32, tag="mu")
    nc.vector.tensor_scalar_mul(mu[:], st_sb[0:1, 0:128], 1.0 / n_nodes)
    ex2 = sb.tile([1, 128], f32, tag="ex2")
    nc.vector.tensor_scalar_mul(ex2[:], st_sb[0:1, 128:256], 1.0 / n_nodes)
    var = sb.tile([1, 128], f32, tag="var")
    nc.vector.tensor_tensor(out=var[:], in0=mu[:], in1=mu[:],
                            op=mybir.AluOpType.mult)
    nc.vector.tensor_tensor(out=var[:], in0=ex2[:], in1=var[:],
                            op=mybir.AluOpType.subtract)
    nc.vector.tensor_scalar_add(var[:], var[:], 1e-5)
    sd = sb.tile([1, 128], f32, tag="sd")
    nc.scalar.sqrt(sd[:], var[:])
    inv = sb.tile([1, 128], f32, tag="invsd")
    nc.vector.reciprocal(inv[:], sd[:])
    nc.vector.tensor_tensor(out=A[:], in0=gamma[:], in1=inv[:],
                            op=mybir.AluOpType.mult)
    nc.vector.tensor_tensor(out=B[:], in0=mu[:], in1=A[:],
                            op=mybir.AluOpType.mult)
    nc.vector.tensor_tensor(out=B[:], in0=beta[:], in1=B[:],
                            op=mybir.AluOpType.subtract)


def bn_elu_tile(nc, sb, z_ap, A, B):
    """h = elu(z*A + B) elementwise; A,B broadcast along partitions."""
    t = sb.tile([P, 128], f32, tag="bn_t")
    nc.vector.tensor_tensor(out=t[:], in0=z_ap, in1=A[:],
                            op=mybir.AluOpType.mult)
    nc.vector.tensor_tensor(out=t[:], in0=t[:], in1=B[:],
                            op=mybir.AluOpType.add)
    m = sb.tile([P, 128], f32, tag="bn_m")
    nc.vector.tensor_scalar_min(m[:], t[:], 0.0)
    e = sb.tile([P, 128], f32, tag="bn_e")
    nc.scalar.activation(e[:], m[:], mybir.ActivationFunctionType.Exp)
    h = sb.tile([P, 128], f32, tag="bn_h")
    nc.vector.tensor_scalar_max(h[:], t[:], 0.0)
    nc.vector.tensor_tensor(out=h[:], in0=h[:], in1=e[:],
                            op=mybir.AluOpType.add)
    nc.vector.tensor_scalar_add(h[:], h[:], -1.0)
    return h


def rowdot(nc, sb, h, w_row, out_col):
    """out_col[p,0] = sum_f h[p,f]*w[f]  (w broadcast along partitions)."""
    t = sb.tile([P, 128], f32, tag="rd_t")
    nc.vector.tensor_tensor(out=t[:], in0=h[:], in1=w_row[:],
                            op=mybir.AluOpType.mult)
    nc.vector.tensor_reduce(out_col[:], t[:], axis=mybir.AxisListType.X,
                            op=mybir.AluOpType.add)




def bcast_vec(nc, sb, ps, ones1, vec_ap, width, tag):
    """Materialize [1,width] -> [P,width] via rank-1 PE matmul."""
    b_ps = ps.tile([P, width], f32, tag="bc_ps", space="PSUM")
    nc.tensor.matmul(b_ps[:], lhsT=ones1[:], rhs=vec_ap, start=True,
                     stop=True)
    b_sb = sb.tile([P, width], f32, tag=tag)
    nc.vector.tensor_copy(b_sb[:], b_ps[:])
    return b_sb


def build_program(cfg, NB):
    N, D, NC, NCORES, TPC = (cfg["N"], cfg["D"], cfg["NC"], cfg["NCORES"],
                             cfg["TPC"])
    NPC = TPC * P                  # nodes per core (padded)
    NTOT = NPC * NCORES            # padded node count
    NBT = int(sum(NB))             # total batches per core
    TROW = 132                     # table2 row: 128 bf16 h + f32 esrc + pad

    nc = bacc.Bacc("TRN2", target_bir_lowering=False, debug=False,
                   num_devices=NCORES)
    # inputs (per core)
    g1s = nc.dram_tensor("g1s", [P, NBT * D], bf16, kind="ExternalInput")
    es1 = nc.dram_tensor("es1", [P, NBT], f32, kind="ExternalInput")
    ed1 = nc.dram_tensor("ed1", [P, TPC], f32, kind="ExternalInput")
    gidx = nc.dram_tensor("gidx", [P, NBT], i32, kind="ExternalInput")
    wmat = nc.dram_tensor("wmat", [D, 2 * D], bf16, kind="ExternalInput")
    wvec = nc.dram_tensor("wvec", [1, 8 * D], f32, kind="ExternalInput")
    wout = nc.dram_tensor("wout", [D, NC], f32, kind="ExternalInput")
    rrows = nc.dram_tensor("rrows", [2, TROW], bf16, kind="ExternalInput")
    out = nc.dram_tensor("out", [NPC, NC], f32, kind="ExternalOutput")

    # internal dram
    t2loc = nc.dram_tensor("t2loc", [NPC, TROW], bf16)
    t2full = nc.dram_tensor("t2full", [NTOT + 2, TROW], bf16,
                            addr_space="Shared")
    ccin = nc.dram_tensor("ccin", [1, 256], f32)
    ccout = nc.dram_tensor("ccout", [1, 256], f32, addr_space="Shared")

    ctx = ExitStack()
    with tile.TileContext(nc, trace_sim=False) as tc, ctx:
        sb = ctx.enter_context(tc.tile_pool(name="sb", bufs=2))
        sbk = ctx.enter_context(tc.tile_pool(name="sbk", bufs=1))  # keepers
        ps = ctx.enter_context(tc.tile_pool(name="ps", bufs=1, space="PSUM"))
        psk = ctx.enter_context(tc.tile_pool(name="psk", bufs=1, space="PSUM"))

        # --- prologue: constants and whole-layer inputs
        from concourse.masks import make_identity
        ident = sbk.tile([P, P], f32)
        make_identity(nc, ident[:])
        W1b = sbk.tile([D, D], bf16)
        nc.sync.dma_start(W1b[:], wmat.ap()[:, 0:D])
        W2b = sbk.tile([D, D], bf16)
        nc.sync.dma_start(W2b[:], wmat.ap()[:, D:2 * D])
        Wo = sbk.tile([D, NC], f32)
        nc.sync.dma_start(Wo[:], wout.ap())
        wv = sbk.tile([1, 8 * D], f32)   # [ws2|wd2|gamma|beta|bout(40)pad|...]
        nc.sync.dma_start(wv[:], wvec.ap())
        ws2 = wv[0:1, 0:D]
        wd2 = wv[0:1, D:2 * D]
        gam = wv[0:1, 2 * D:3 * D]
        bet = wv[0:1, 3 * D:4 * D]
        bo = wv[0:1, 4 * D:4 * D + NC]
        ed1_t = sbk.tile([P, TPC], f32)
        nc.sync.dma_start(ed1_t[:], ed1.ap())
        gidx_t = sbk.tile([P, NBT], i32)
        nc.sync.dma_start(gidx_t[:], gidx.ap())
        ones = sbk.tile([P, 1], bf16)
        nc.vector.memset(ones[:], 1.0)
        ones1 = sbk.tile([1, P], f32)
        nc.vector.memset(ones1[:], 1.0)
        stash = sbk.tile([P, TPC * D], f32)
        ed2_t = sbk.tile([P, TPC], f32)
        esrc2_c = sbk.tile([P, 1], f32)
        st_sb = sbk.tile([1, 256], f32)
        A1 = sbk.tile([1, 128], f32)
        B1 = sbk.tile([1, 128], f32)
        A2 = sbk.tile([1, 128], f32)
        B2 = sbk.tile([1, 128], f32)
        rr = sbk.tile([2, TROW], bf16)
        nc.sync.dma_start(rr[:], rrows.ap())
        st_z = psk.tile([1, 128], f32, space="PSUM")
        st_z2 = psk.tile([1, 128], f32, space="PSUM")
        ws2b = bcast_vec(nc, sbk, ps, ones1, ws2, D, "ws2b")
        wd2b = bcast_vec(nc, sbk, ps, ones1, wd2, D, "wd2b")
        bob = bcast_vec(nc, sbk, ps, ones1, bo, NC, "bob")
        tc.strict_bb_all_engine_barrier()

        # ---------------- layer 1 ----------------
        off = 0
        for j in range(TPC):
            nb = NB[j]
            G1 = sb.tile([P, nb * D], bf16, tag="G1")
            nc.sync.dma_start(G1[:], g1s.ap()[:, off * D:(off + nb) * D])
            es1_t = sb.tile([P, nb], f32, tag="es1")
            nc.sync.dma_start(es1_t[:], es1.ap()[:, off:off + nb])
            den = sb.tile([P, 1], f32, tag="den")
            p = leaky_exp(nc, sb, es1_t[:], ed1_t[:, j:j + 1], nb, den)
            z_ps = ps.tile([P, D], f32, tag="z_ps", space="PSUM")
            aggregate(nc, sb, ps, G1[:].rearrange("p (b f) -> p b f", f=D),
                      p, den, nb, ident, W1b, z_ps)
            nc.vector.tensor_copy(stash[:, j * D:(j + 1) * D], z_ps[:])
            stats_accum(nc, sb, stash[:, j * D:(j + 1) * D], ones, st_z,
                        st_z2, j, j == TPC - 1)
            off += nb

        # BN1 stats allreduce
        tc.strict_bb_all_engine_barrier()
        nc.vector.tensor_copy(st_sb[0:1, 0:128], st_z[:])
        nc.vector.tensor_copy(st_sb[0:1, 128:256], st_z2[:])
        nc.sync.dma_start(ccin.ap(), st_sb[:])
        tc.strict_bb_all_engine_barrier()
        nc.gpsimd.collective_compute(
            "AllReduce", mybir.AluOpType.add,
            replica_groups=[list(range(NCORES))],
            ins=[ccin.ap()], outs=[ccout.ap()])
        tc.strict_bb_all_engine_barrier()
        nc.sync.dma_start(st_sb[:], ccout.ap())
        tc.strict_bb_all_engine_barrier()
        bn_consts(nc, sb, st_sb, gam, bet, N, A1, B1)
        A1b = bcast_vec(nc, sbk, ps, ones1, A1[:], D, "A1b")
        B1b = bcast_vec(nc, sbk, ps, ones1, B1[:], D, "B1b")
        tc.strict_bb_all_engine_barrier()

        # ---------------- build table2 ----------------
        for j in range(TPC):
            h1 = bn_elu_tile(nc, sb, stash[:, j * D:(j + 1) * D], A1b, B1b)
            rowdot(nc, sb, h1, ws2b, esrc2_c[:])
            rowdot(nc, sb, h1, wd2b, ed2_t[:, j:j + 1])
            row = sb.tile([P, TROW], bf16, tag="row")
            nc.vector.tensor_copy(row[:, 0:D], h1[:])
            nc.vector.tensor_copy(row[:, D:D + 2].bitcast(f32), esrc2_c[:])
            nc.vector.memset(row[:, D + 2:TROW], 0.0)
            nc.sync.dma_start(t2loc.ap()[j * P:(j + 1) * P, :], row[:])
        tc.strict_bb_all_engine_barrier()
        nc.gpsimd.collective_compute(
            "AllGather", mybir.AluOpType.bypass,
            replica_groups=[list(range(NCORES))],
            ins=[t2loc.ap()], outs=[t2full.ap()[0:NTOT, :]])
        nc.sync.dma_start(t2full.ap()[NTOT:NTOT + 2, :], rr[:])
        tc.strict_bb_all_engine_barrier()

        # ---------------- layer 2 ----------------
        off = 0
        for j in range(TPC):
            nb = NB[j]
            G2 = sb.tile([P, nb * TROW], bf16, tag="G2")
            for b in range(nb):
                nc.gpsimd.indirect_dma_start(
                    out=G2[:, b * TROW:(b + 1) * TROW],
                    out_offset=None,
                    in_=t2full.ap(),
                    in_offset=IndirectOffsetOnAxis(
                        ap=gidx_t[:, off + b:off + b + 1], axis=0))
            G23 = G2[:].rearrange("p (b r) -> p b r", r=TROW)
            es2 = G2[:].bitcast(f32).rearrange(
                "p (b r) -> p b r", r=TROW // 2)[:, :, D // 2]
            den = sb.tile([P, 1], f32, tag="den")
            p = leaky_exp(nc, sb, es2, ed2_t[:, j:j + 1], nb, den)
            z_ps = ps.tile([P, D], f32, tag="z_ps", space="PSUM")
            aggregate(nc, sb, ps, G23[:, :, 0:D], p, den, nb, ident, W2b,
                      z_ps)
            nc.vector.tensor_copy(stash[:, j * D:(j + 1) * D], z_ps[:])
            stats_accum(nc, sb, stash[:, j * D:(j + 1) * D], ones, st_z,
                        st_z2, j, j == TPC - 1)
            off += nb

        # BN2 stats allreduce
        tc.strict_bb_all_engine_barrier()
        nc.vector.tensor_copy(st_sb[0:1, 0:128], st_z[:])
        nc.vector.tensor_copy(st_sb[0:1, 128:256], st_z2[:])
        nc.sync.dma_start(ccin.ap(), st_sb[:])
        tc.strict_bb_all_engine_barrier()
        nc.gpsimd.collective_compute(
            "AllReduce", mybir.AluOpType.add,
            replica_groups=[list(range(NCORES))],
            ins=[ccin.ap()], outs=[ccout.ap()])
        tc.strict_bb_all_engine_barrier()
        nc.sync.dma_start(st_sb[:], ccout.ap())
        tc.strict_bb_all_engine_barrier()
        bn_consts(nc, sb, st_sb, gam, bet, N, A2, B2)
        A2b = bcast_vec(nc, sbk, ps, ones1, A2[:], D, "A2b")
        B2b = bcast_vec(nc, sbk, ps, ones1, B2[:], D, "B2b")
        tc.strict_bb_all_engine_barrier()

        # ---------------- head ----------------
        for j in range(TPC):
            h2 = bn_elu_tile(nc, sb, stash[:, j * D:(j + 1) * D], A2b, B2b)
            h2T_ps = ps.tile([P, P], f32, tag="aggT_ps", space="PSUM")
            nc.tensor.transpose(out=h2T_ps[:], in_=h2[:], identity=ident[:])
            h2T = sb.tile([P, P], f32, tag="h2T")
            nc.vector.tensor_copy(h2T[:], h2T_ps[:])
            o_ps = ps.tile([P, NC], f32, tag="z_ps", space="PSUM")
            nc.tensor.matmul(o_ps[:], lhsT=h2T[:], rhs=Wo[:], start=True,
                             stop=True)
            o_t = sb.tile([P, NC], f32, tag="o_t")
            nc.vector.tensor_tensor(out=o_t[:], in0=o_ps[:], in1=bob[:],
                                    op=mybir.AluOpType.add)
            nc.sync.dma_start(out.ap()[j * P:(j + 1) * P, :], o_t[:])
    nc.compile()
    return nc


def prepare(cfg, x, edge_index, W1, a_src1, a_dst1, W2, a_src2, a_dst2,
            gamma, beta, Wout, bout):
    """Host-side graph routing + input packing. Returns (in_maps, meta)."""
    N, D, NC, NCORES, TPC = (cfg["N"], cfg["D"], cfg["NC"], cfg["NCORES"],
                             cfg["TPC"])
    NPC = TPC * P
    NTOT = NPC * NCORES

    src = np.concatenate([edge_index[0], np.arange(N)]).astype(np.int64)
    dst = np.concatenate([edge_index[1], np.arange(N)]).astype(np.int64)
    deg = np.bincount(dst, minlength=N)

    order = np.argsort(-deg, kind="stable")  # degree-sorted original ids
    # global tile t gets order[128t:128t+128]; tile t -> core t%NCORES,
    # slot t//NCORES. node table position = core*NPC + slot*128 + p
    tpos = np.full(N, -1, np.int64)
    ntile = NTOT // P
    tposs = (np.arange(ntile) % NCORES) * NPC + (np.arange(ntile) // NCORES) * P
    for t in range((N + P - 1) // P):
        ids = order[t * P:(t + 1) * P]
        tpos[ids] = tposs[t] + np.arange(len(ids))

    # per-slot batch counts: NB[j] = max over cores of max deg in that tile
    degp = np.zeros(NTOT, np.int64)
    degp[tpos[order]] = deg[order]
    degp = degp.reshape(NCORES, TPC, P)
    NB = np.maximum(degp.max(axis=(0, 2)), 1).astype(np.int64)  # [TPC]
    NBT = int(NB.sum())

    # edge lists grouped by destination table position
    eorder = np.argsort(tpos[dst], kind="stable")
    srcs = src[eorder]                      # srcs grouped by dst tpos
    starts = np.zeros(NTOT + 1, np.int64)
    counts = np.bincount(tpos[dst], minlength=NTOT)
    starts[1:] = np.cumsum(counts)

    ws1 = (W1 @ a_src1).astype(np.float32)
    wd1 = (W1 @ a_dst1).astype(np.float32)
    es1_node = (x @ ws1).astype(np.float32)
    ed1_node = (x @ wd1).astype(np.float32)
    x_bf = x.astype(ml_dtypes.bfloat16)

    ws2 = (W2 @ a_src2).astype(np.float32)
    wd2 = (W2 @ a_dst2).astype(np.float32)

    # slot tables per core
    boff = np.zeros(TPC + 1, np.int64)
    boff[1:] = np.cumsum(NB)

    # vectorized slot filling: slot (tp, b) for b < deg[tp] takes edge
    # srcs[starts[tp] + b]; build a [NTOT, NBmax]-shaped scatter via arange.
    RROW, R2ROW = NTOT, NTOT + 1
    # per-node slot base: node at (c, j, p) uses batch offsets boff[j]
    jidx = (np.arange(NTOT) % NPC) // P          # tile slot per tpos
    slot_base = boff[jidx]                        # [NTOT]
    eslot = np.empty(len(srcs), np.int64)         # flat slot per edge
    # position of each edge within its dst segment:
    within = np.arange(len(srcs)) - np.repeat(starts[:-1], counts)
    etp = np.repeat(np.arange(NTOT), counts)      # dst tpos per edge
    eslot = (etp % NPC) % P  # partition
    ecore = etp // NPC
    ecol = slot_base[etp] + within                # batch column
    epart = (etp % NPC) % P

    gx_all = np.full((NCORES, P, NBT), RROW, np.int32)
    e1_all = np.full((NCORES, P, NBT), NEG, np.float32)
    g1_all = np.zeros((NCORES, P, NBT, D), ml_dtypes.bfloat16)
    gx_all[ecore, epart, ecol] = tpos[srcs].astype(np.int32)
    e1_all[ecore, epart, ecol] = es1_node[srcs]
    g1_all[ecore, epart, ecol] = x_bf[srcs]
    # pad nodes (tpos slots with zero degree): neutral first edge
    padm = (counts == 0)
    ptp = np.nonzero(padm)[0]
    gx_all[ptp // NPC, (ptp % NPC) % P, slot_base[ptp]] = R2ROW
    e1_all[ptp // NPC, (ptp % NPC) % P, slot_base[ptp]] = 0.0

    # edst per (core, p, j)
    ed1_pos = np.zeros(NTOT, np.float32)
    ed1_pos[tpos] = ed1_node
    ed1_all = ed1_pos.reshape(NCORES, TPC, P).transpose(0, 2, 1)

    wvec = np.zeros((1, 8 * D), np.float32)
    wvec[0, 0:D] = ws2
    wvec[0, D:2 * D] = wd2
    wvec[0, 2 * D:3 * D] = gamma
    wvec[0, 3 * D:4 * D] = beta
    wvec[0, 4 * D:4 * D + NC] = bout
    rrows = np.zeros((2, 132), ml_dtypes.bfloat16)
    rr_raw = rrows.view(np.uint16)
    neg_bits = int(np.float32(NEG).view(np.uint32))
    rr_raw[0, D] = neg_bits & 0xFFFF
    rr_raw[0, D + 1] = neg_bits >> 16
    wmat = np.ascontiguousarray(np.concatenate(
        [W1.astype(ml_dtypes.bfloat16), W2.astype(ml_dtypes.bfloat16)],
        axis=1))
    in_maps = []
    for c in range(NCORES):
        in_maps.append({
            "g1s": np.ascontiguousarray(g1_all[c].reshape(P, NBT * D)),
            "es1": np.ascontiguousarray(e1_all[c]),
            "ed1": np.ascontiguousarray(ed1_all[c]),
            "gidx": np.ascontiguousarray(gx_all[c]),
            "wmat": wmat,
            "wvec": wvec,
            "wout": np.ascontiguousarray(Wout.astype(np.float32)),
            "rrows": rr_raw.view(ml_dtypes.bfloat16),
        })
    meta = dict(NB=NB, tpos=tpos)
    return in_maps, meta


_CACHE = {}


def kernel(x, edge_index, W1, a_src1, a_dst1, b1, W2, a_src2, a_dst2, b2,
           gamma, beta, Wout, bout, cfg=None):
    cfg = cfg or CFG_FULL
    x = np.asarray(x, np.float32)
    edge_index = np.asarray(edge_index)
    args = [np.asarray(a, np.float32) for a in
            (W1, a_src1, a_dst1, W2, a_src2, a_dst2, gamma, beta, Wout, bout)]
    (W1, a_src1, a_dst1, W2, a_src2, a_dst2, gamma, beta, Wout, bout) = args

    in_maps, meta = prepare(cfg, x, edge_index, W1, a_src1, a_dst1,
                            W2, a_src2, a_dst2, gamma, beta, Wout, bout)
    key = (cfg["N"], tuple(meta["NB"]))
    if key not in _CACHE:
        _CACHE[key] = build_program(cfg, meta["NB"])
    nc = _CACHE[key]
    res = run_bass_kernel_spmd(nc, in_maps, list(range(cfg["NCORES"])))

    N, NC, NCORES, NPC = cfg["N"], cfg["NC"], cfg["NCORES"], cfg["TPC"] * P
    full = np.zeros((NCORES * NPC, NC), np.float32)
    for c in range(NCORES):
        full[c * NPC:(c + 1) * NPC] = res.results[c]["out"]
    return np.ascontiguousarray(full[meta["tpos"]])



# revision 2
# speedup vs baseline: 1.0359x; 1.0359x over previous
"""GAT 2-layer + BN/ELU + linear head on 8 Trainium2 cores — v2.

Layout: nodes degree-sorted, tiled 128 dsts/tile, 98 tiles/core (dst-owner
sharding per spec hint). Layer-1 src features host-routed per edge (halo
exchange at input staging); layer-2 src features gathered on device from an
AllGathered node table. Self-loop edges of layer 2 are served from an
on-chip stash of h1 rows instead of costing gather descriptors.

Engine split per tile: GpSimd runs the per-batch indirect row gathers (L2);
Scalar(ACT) computes leaky+exp scores with the per-dst bias folded in and
accumulates softmax denominators and BN statistics via accum_out; Vector
does one fused multiply over all batches plus one strided reduce; Tensor
does the transpose + weight matmuls on a transposed z path ([feat, dst])
so BN affine is a per-partition scale/bias on ACT and the head needs no
final transpose (output written as [NC, nodes]).
"""
import sys
sys.path.insert(0, "/opt/trn_rl_repo")
from contextlib import ExitStack

import numpy as np
import ml_dtypes

import concourse.mybir as mybir
import concourse.tile as tile
from concourse import bacc
from concourse.bass import IndirectOffsetOnAxis
from concourse.bass_utils import run_bass_kernel_spmd

P = 128
f32 = mybir.dt.float32
bf16 = mybir.dt.bfloat16
i32 = mybir.dt.int32

NEG = -1e30  # score sentinel: exp(leaky(NEG + finite)) == 0
TROW = 132   # table2 row: 128 bf16 h + f32 esrc (2 slots) + 2 pad

CFG_FULL = dict(N=100000, D=128, NC=40, NCORES=8, TPC=98)

Act = mybir.ActivationFunctionType
Alu = mybir.AluOpType


def build_program(cfg, NB1, NB2):
    N, D, NC, NCORES, TPC = (cfg["N"], cfg["D"], cfg["NC"], cfg["NCORES"],
                             cfg["TPC"])
    NPC = TPC * P
    NTOT = NPC * NCORES
    NBT1 = int(sum(NB1))
    NBT2 = int(sum(NB2))

    nc = bacc.Bacc("TRN2", target_bir_lowering=False, debug=False,
                   num_devices=NCORES)
    # inputs
    g1s = nc.dram_tensor("g1s", [P, NBT1 * D], bf16, kind="ExternalInput")
    es1 = nc.dram_tensor("es1", [P, NBT1], f32, kind="ExternalInput")
    ed1 = nc.dram_tensor("ed1", [P, TPC], f32, kind="ExternalInput")
    gidx = nc.dram_tensor("gidx", [P, NBT2], i32, kind="ExternalInput")
    nmask = nc.dram_tensor("nmask", [P, TPC], f32, kind="ExternalInput")
    wmat = nc.dram_tensor("wmat", [D, 2 * D], bf16, kind="ExternalInput")
    wcols = nc.dram_tensor("wcols", [D, 8], f32, kind="ExternalInput")
    woutb = nc.dram_tensor("woutb", [D, NC], bf16, kind="ExternalInput")
    rrows = nc.dram_tensor("rrows", [2, TROW], bf16, kind="ExternalInput")
    out = nc.dram_tensor("out", [NC, NPC], f32, kind="ExternalOutput")

    # internal dram
    t2loc = nc.dram_tensor("t2loc", [NPC, TROW], bf16)
    t2full = nc.dram_tensor("t2full", [NTOT + 2, TROW], bf16,
                            addr_space="Shared")
    ccin = nc.dram_tensor("ccin", [P, 2], f32)
    ccout = nc.dram_tensor("ccout", [P, 2], f32, addr_space="Shared")

    ctx = ExitStack()
    with tile.TileContext(nc, trace_sim=False) as tc, ctx:
        sb = ctx.enter_context(tc.tile_pool(name="sb", bufs=3))
        sbk = ctx.enter_context(tc.tile_pool(name="sbk", bufs=1))
        ps = ctx.enter_context(tc.tile_pool(name="ps", bufs=2, space="PSUM"))
        psk = ctx.enter_context(tc.tile_pool(name="psk", bufs=1,
                                             space="PSUM"))

        # --- prologue
        from concourse.masks import make_identity
        identf = sbk.tile([P, P], f32)
        make_identity(nc, identf[:])
        ident = sbk.tile([P, P], bf16)
        nc.vector.tensor_copy(ident[:], identf[:])
        W1b = sbk.tile([D, D], bf16)
        nc.sync.dma_start(W1b[:], wmat.ap()[:, 0:D])
        W2b = sbk.tile([D, D], bf16)
        nc.sync.dma_start(W2b[:], wmat.ap()[:, D:2 * D])
        Wo = sbk.tile([D, NC], bf16)
        nc.sync.dma_start(Wo[:], woutb.ap())
        wc = sbk.tile([D, 8], f32)
        nc.sync.dma_start(wc[:], wcols.ap())
        esed_rhs = sbk.tile([D, 2], bf16)     # [ws2 | wd2] as PE rhs
        nc.vector.tensor_copy(esed_rhs[:], wc[:, 0:2])
        gam = wc[:, 2:3]
        bet = wc[:, 3:4]
        boutc = wc[0:NC, 4:5]
        ed1_t = sbk.tile([P, TPC], f32)
        nc.sync.dma_start(ed1_t[:], ed1.ap())
        es1_t = sbk.tile([P, NBT1], f32)
        nc.sync.dma_start(es1_t[:], es1.ap())
        gidx_t = sbk.tile([P, NBT2], i32)
        nc.sync.dma_start(gidx_t[:], gidx.ap())
        nmask_t = sbk.tile([P, TPC], f32)
        nc.sync.dma_start(nmask_t[:], nmask.ap())
        rr = sbk.tile([2, TROW], bf16)
        nc.sync.dma_start(rr[:], rrows.ap())
        nc.sync.dma_start(t2full.ap()[NTOT:NTOT + 2, :], rr[:])

        stash = sbk.tile([P, TPC * D], f32)     # zT per tile [feat, dst]
        stashR = sbk.tile([P, TPC * D], bf16)   # h1 rows per tile [dst, feat]
        esed_t = sbk.tile([P, 2 * TPC], f32)    # es2|ed2 cols per tile
        sz = sbk.tile([P, TPC], f32)            # per-tile sum z (per feat)
        sq = sbk.tile([P, TPC], f32)            # per-tile sum z^2
        stA = sbk.tile([P, 1], f32)
        stB = sbk.tile([P, 1], f32)
        st_sb = sbk.tile([P, 2], f32)
        sqscr = sbk.tile([P, D], bf16)          # square scratch
        tc.strict_bb_all_engine_barrier()

        def agg_z(j, blocks, p_bf, den, Wb):
            """blocks: [P, nblk, D] bf16 view(s) list [(ap3, col0, n)].
            aggT = sum_b (pn_b * G_b)^T accumulated on PE; returns zT psum."""
            nblk = p_bf.shape[1]
            inv = sb.tile([P, 1], f32, tag="inv")
            nc.vector.reciprocal(inv[:], den[:])
            pn = sb.tile([P, nblk], bf16, tag="pn")
            nc.vector.tensor_scalar_mul(pn[:], p_bf[:], inv[:])
            Gw = sb.tile([P, nblk * D], bf16, tag="Gw")
            for ap3, c0, n in blocks:
                nc.vector.tensor_tensor(
                    out=Gw[:].rearrange("p (b f) -> p b f", f=D)[:, c0:c0 + n],
                    in0=ap3,
                    in1=pn[:, c0:c0 + n].unsqueeze(2).to_broadcast(
                        [P, n, D]),
                    op=Alu.mult)
            aggT_ps = ps.tile([P, P], f32, tag="aggT_ps", space="PSUM")
            for b in range(nblk):
                nc.tensor.matmul(aggT_ps[:], lhsT=Gw[:, b * D:(b + 1) * D],
                                 rhs=ident[:], start=(b == 0),
                                 stop=(b == nblk - 1))
            aggT = sb.tile([P, P], bf16, tag="aggT")
            nc.vector.tensor_copy(aggT[:], aggT_ps[:])
            zT_ps = ps.tile([P, D], f32, tag="zT_ps", space="PSUM")
            nc.tensor.matmul(zT_ps[:], lhsT=Wb[:], rhs=aggT[:], start=True,
                             stop=True)
            return zT_ps

        def stash_stats(j, zT_ps):
            """stash zT and accumulate per-feature sum/sumsq columns."""
            nc.scalar.activation(stash[:, j * D:(j + 1) * D], zT_ps[:],
                                 Act.Copy, accum_out=sz[:, j:j + 1])
            nc.scalar.activation(sqscr[:], zT_ps[:], Act.Square,
                                 accum_out=sq[:, j:j + 1])

        # ---------------- layer 1 ----------------
        off = 0
        for j in range(TPC):
            nb = int(NB1[j])
            G1 = sb.tile([P, nb * D], bf16, tag="G1")
            nc.sync.dma_start(G1[:], g1s.ap()[:, off * D:(off + nb) * D])
            t = sb.tile([P, nb], f32, tag="t")
            nc.scalar.activation(t[:], es1_t[:, off:off + nb], Act.Prelu,
                                 bias=ed1_t[:, j:j + 1], alpha=0.2)
            den = sb.tile([P, 1], f32, tag="den")
            p_bf = sb.tile([P, nb], bf16, tag="p_bf")
            nc.scalar.activation(p_bf[:], t[:], Act.Exp, accum_out=den[:])
            zT_ps = agg_z(j, [(G1[:].rearrange("p (b f) -> p b f", f=D), 0,
                               nb)], p_bf, den, W1b)
            stash_stats(j, zT_ps)
            off += nb

        # BN1 stats allreduce
        nc.vector.tensor_reduce(st_sb[:, 0:1], sz[:], axis=mybir.AxisListType.X,
                                op=Alu.add)
        nc.vector.tensor_reduce(st_sb[:, 1:2], sq[:], axis=mybir.AxisListType.X,
                                op=Alu.add)
        nc.sync.dma_start(ccin.ap(), st_sb[:])
        tc.strict_bb_all_engine_barrier()
        nc.gpsimd.collective_compute(
            "AllReduce", Alu.add, replica_groups=[list(range(NCORES))],
            ins=[ccin.ap()], outs=[ccout.ap()])
        tc.strict_bb_all_engine_barrier()
        nc.sync.dma_start(st_sb[:], ccout.ap())

        def bn_consts(A, B):
            """A = gamma/sqrt(var+eps), B = beta - mu*A, all [P,1] f32."""
            mu = sb.tile([P, 1], f32, tag="mu")
            nc.vector.tensor_scalar_mul(mu[:], st_sb[:, 0:1], 1.0 / N)
            ex2 = sb.tile([P, 1], f32, tag="ex2")
            nc.vector.tensor_scalar_mul(ex2[:], st_sb[:, 1:2], 1.0 / N)
            var = sb.tile([P, 1], f32, tag="var")
            nc.vector.tensor_tensor(out=var[:], in0=mu[:], in1=mu[:],
                                    op=Alu.mult)
            nc.vector.tensor_tensor(out=var[:], in0=ex2[:], in1=var[:],
                                    op=Alu.subtract)
            nc.vector.tensor_scalar_add(var[:], var[:], 1e-5)
            sd = sb.tile([P, 1], f32, tag="sd")
            nc.scalar.activation(sd[:], var[:], Act.Sqrt)
            nc.vector.reciprocal(sd[:], sd[:])
            nc.vector.tensor_tensor(out=A[:], in0=gam, in1=sd[:], op=Alu.mult)
            nc.vector.tensor_tensor(out=B[:], in0=mu[:], in1=A[:], op=Alu.mult)
            nc.vector.tensor_tensor(out=B[:], in0=bet, in1=B[:],
                                    op=Alu.subtract)

        bn_consts(stA, stB)

        def bn_elu(j, A, B, tag):
            """h^T = elu(z^T*A + B) from stash, [P(feat), 128dst] bf16.
            elu(t) = max(t,0) + exp(min(t,0)) - 1."""
            t = sb.tile([P, D], f32, tag=tag + "_t")
            nc.scalar.activation(t[:], stash[:, j * D:(j + 1) * D],
                                 Act.Identity, bias=B[:], scale=A[:])
            u = sb.tile([P, D], f32, tag=tag + "_u")
            nc.vector.tensor_scalar_min(u[:], t[:], 0.0)
            e = sb.tile([P, D], f32, tag=tag + "_e")
            nc.scalar.activation(e[:], u[:], Act.Exp)
            r = sb.tile([P, D], f32, tag=tag + "_r")
            nc.vector.tensor_scalar_max(r[:], t[:], 0.0)
            nc.vector.tensor_tensor(out=r[:], in0=r[:], in1=e[:], op=Alu.add)
            h = sb.tile([P, D], bf16, tag=tag + "_h")
            nc.vector.tensor_scalar_add(h[:], r[:], -1.0)
            return h

        # ---------------- build table2 ----------------
        for j in range(TPC):
            h1T = bn_elu(j, stA, stB, "b1")
            esed_ps = psk.tile([P, 2], f32, tag="esed_ps", space="PSUM")
            nc.tensor.matmul(esed_ps[:], lhsT=h1T[:], rhs=esed_rhs[:],
                             start=True, stop=True)
            nc.vector.tensor_copy(esed_t[:, 2 * j:2 * j + 2], esed_ps[:])
            rows_ps = psk.tile([P, P], f32, tag="rows_ps", space="PSUM")
            nc.tensor.matmul(rows_ps[:], lhsT=h1T[:], rhs=ident[:],
                             start=True, stop=True)
            nc.scalar.activation(stashR[:, j * D:(j + 1) * D], rows_ps[:],
                                 Act.Copy)
            row = sb.tile([P, TROW], bf16, tag="row")
            nc.vector.tensor_copy(row[:, 0:D], stashR[:, j * D:(j + 1) * D])
            nc.vector.tensor_copy(row[:, D:D + 2].bitcast(f32),
                                  esed_ps[:, 0:1])
            nc.vector.memset(row[:, D + 2:TROW], 0.0)
            nc.sync.dma_start(t2loc.ap()[j * P:(j + 1) * P, :], row[:])
        # mask self scores of pad nodes
        nc.vector.tensor_tensor(
            out=esed_t[:].rearrange("p (j two) -> p j two", two=2)[:, :, 0],
            in0=esed_t[:].rearrange("p (j two) -> p j two", two=2)[:, :, 0],
            in1=nmask_t[:], op=Alu.add)
        tc.strict_bb_all_engine_barrier()
        nc.gpsimd.collective_compute(
            "AllGather", Alu.bypass, replica_groups=[list(range(NCORES))],
            ins=[t2loc.ap()], outs=[t2full.ap()[0:NTOT, :]])
        tc.strict_bb_all_engine_barrier()

        # ---------------- layer 2 ----------------
        off = 0
        for j in range(TPC):
            nb = int(NB2[j])
            nblk = nb + 1
            G2 = sb.tile([P, nb * TROW], bf16, tag="G2")
            for b in range(nb):
                nc.gpsimd.indirect_dma_start(
                    out=G2[:, b * TROW:(b + 1) * TROW],
                    out_offset=None,
                    in_=t2full.ap(),
                    in_offset=IndirectOffsetOnAxis(
                        ap=gidx_t[:, off + b:off + b + 1], axis=0))
            es2g = G2[:].bitcast(f32).rearrange(
                "p (b r) -> p b r", r=TROW // 2)[:, :, D // 2]
            t = sb.tile([P, nblk], f32, tag="t")
            nc.scalar.activation(t[:, 0:1], esed_t[:, 2 * j:2 * j + 1],
                                 Act.Prelu, bias=esed_t[:, 2 * j + 1:2 * j + 2],
                                 alpha=0.2)
            nc.scalar.activation(t[:, 1:nblk], es2g, Act.Prelu,
                                 bias=esed_t[:, 2 * j + 1:2 * j + 2], alpha=0.2)
            den = sb.tile([P, 1], f32, tag="den")
            p_bf = sb.tile([P, nblk], bf16, tag="p_bf")
            nc.scalar.activation(p_bf[:], t[:], Act.Exp, accum_out=den[:])
            blocks = [
                (stashR[:, j * D:(j + 1) * D].unsqueeze(1), 0, 1),
                (G2[:].rearrange("p (b r) -> p b r", r=TROW)[:, :, 0:D], 1,
                 nb),
            ]
            zT_ps = agg_z(j, blocks, p_bf, den, W2b)
            stash_stats(j, zT_ps)
            off += nb

        # BN2 stats allreduce
        nc.vector.tensor_reduce(st_sb[:, 0:1], sz[:], axis=mybir.AxisListType.X,
                                op=Alu.add)
        nc.vector.tensor_reduce(st_sb[:, 1:2], sq[:], axis=mybir.AxisListType.X,
                                op=Alu.add)
        nc.sync.dma_start(ccin.ap(), st_sb[:])
        tc.strict_bb_all_engine_barrier()
        nc.gpsimd.collective_compute(
            "AllReduce", Alu.add, replica_groups=[list(range(NCORES))],
            ins=[ccin.ap()], outs=[ccout.ap()])
        tc.strict_bb_all_engine_barrier()
        nc.sync.dma_start(st_sb[:], ccout.ap())
        bn_consts(stA, stB)

        # ---------------- head ----------------
        for j in range(TPC):
            h2T = bn_elu(j, stA, stB, "b2")
            oT_ps = ps.tile([NC, P], f32, tag="zT_ps", space="PSUM")
            nc.tensor.matmul(oT_ps[:], lhsT=Wo[:], rhs=h2T[:], start=True,
                             stop=True)
            o_t = sb.tile([NC, P], f32, tag="o_t")
            nc.scalar.activation(o_t[:], oT_ps[:], Act.Identity, bias=boutc)
            nc.sync.dma_start(out.ap()[:, j * P:(j + 1) * P], o_t[:])
    nc.compile()
    return nc


def prepare(cfg, x, edge_index, W1, a_src1, a_dst1, W2, a_src2, a_dst2,
            gamma, beta, Wout, bout):
    """Host-side graph routing + input packing. Returns (in_maps, meta)."""
    N, D, NC, NCORES, TPC = (cfg["N"], cfg["D"], cfg["NC"], cfg["NCORES"],
                             cfg["TPC"])
    NPC = TPC * P
    NTOT = NPC * NCORES

    src = np.concatenate([edge_index[0], np.arange(N)]).astype(np.int64)
    dst = np.concatenate([edge_index[1], np.arange(N)]).astype(np.int64)
    deg = np.bincount(dst, minlength=N)          # includes self-loop

    order = np.argsort(-deg, kind="stable")
    tpos = np.full(N, -1, np.int64)
    ntile = NTOT // P
    tposs = (np.arange(ntile) % NCORES) * NPC + (np.arange(ntile) // NCORES) * P
    for t in range((N + P - 1) // P):
        ids = order[t * P:(t + 1) * P]
        tpos[ids] = tposs[t] + np.arange(len(ids))

    degp = np.zeros(NTOT, np.int64)
    degp[tpos[order]] = deg[order]
    degp3 = degp.reshape(NCORES, TPC, P)
    NB1 = np.maximum(degp3.max(axis=(0, 2)), 1).astype(np.int64)   # [TPC]
    NB2 = np.maximum(NB1 - 1, 1).astype(np.int64)
    NBT1 = int(NB1.sum())
    NBT2 = int(NB2.sum())

    # ---- layer 1 slots (edges incl self-loops), grouped by dst tpos
    eorder = np.argsort(tpos[dst], kind="stable")
    srcs = src[eorder]
    counts = np.bincount(tpos[dst], minlength=NTOT)
    starts = np.zeros(NTOT + 1, np.int64)
    starts[1:] = np.cumsum(counts)

    ws1 = (W1 @ a_src1).astype(np.float32)
    wd1 = (W1 @ a_dst1).astype(np.float32)
    es1_node = (x @ ws1).astype(np.float32)
    ed1_node = (x @ wd1).astype(np.float32)
    x_bf = x.astype(ml_dtypes.bfloat16)
    ws2 = (W2 @ a_src2).astype(np.float32)
    wd2 = (W2 @ a_dst2).astype(np.float32)

    boff1 = np.zeros(TPC + 1, np.int64)
    boff1[1:] = np.cumsum(NB1)
    jidx = (np.arange(NTOT) % NPC) // P
    slot1 = boff1[jidx]

    within = np.arange(len(srcs)) - np.repeat(starts[:-1], counts)
    etp = np.repeat(np.arange(NTOT), counts)
    ecore = etp // NPC
    epart = (etp % NPC) % P
    ecol1 = slot1[etp] + within

    RROW, R2ROW = NTOT, NTOT + 1
    e1_all = np.full((NCORES, P, NBT1), NEG, np.float32)
    g1_all = np.zeros((NCORES, P, NBT1, D), ml_dtypes.bfloat16)
    e1_all[ecore, epart, ecol1] = es1_node[srcs]
    g1_all[ecore, epart, ecol1] = x_bf[srcs]
    padm = (counts == 0)
    ptp = np.nonzero(padm)[0]
    e1_all[ptp // NPC, (ptp % NPC) % P, slot1[ptp]] = 0.0

    ed1_pos = np.zeros(NTOT, np.float32)
    ed1_pos[tpos] = ed1_node
    ed1_all = ed1_pos.reshape(NCORES, TPC, P).transpose(0, 2, 1)

    # ---- layer 2 slots (edges WITHOUT self-loops)
    src2 = edge_index[0].astype(np.int64)
    dst2 = edge_index[1].astype(np.int64)
    eorder2 = np.argsort(tpos[dst2], kind="stable")
    srcs2 = src2[eorder2]
    counts2 = np.bincount(tpos[dst2], minlength=NTOT)
    starts2 = np.zeros(NTOT + 1, np.int64)
    starts2[1:] = np.cumsum(counts2)

    boff2 = np.zeros(TPC + 1, np.int64)
    boff2[1:] = np.cumsum(NB2)
    slot2 = boff2[jidx]

    within2 = np.arange(len(srcs2)) - np.repeat(starts2[:-1], counts2)
    etp2 = np.repeat(np.arange(NTOT), counts2)
    ecore2 = etp2 // NPC
    epart2 = (etp2 % NPC) % P
    ecol2 = slot2[etp2] + within2

    gx_all = np.full((NCORES, P, NBT2), RROW, np.int32)
    gx_all[ecore2, epart2, ecol2] = tpos[srcs2].astype(np.int32)
    # pad nodes: neutral first slot (h=0, es=0) keeps denom > 0 with a
    # masked self column
    gx_all[ptp // NPC, (ptp % NPC) % P, slot2[ptp]] = R2ROW
    nmask_all = np.zeros((NCORES, P, TPC), np.float32)
    nm = nmask_all.reshape(NCORES, P, TPC)
    nm[ptp // NPC, (ptp % NPC) % P, (ptp % NPC) // P] = NEG

    wcols = np.zeros((D, 8), np.float32)
    wcols[:, 0] = ws2
    wcols[:, 1] = wd2
    wcols[:, 2] = gamma
    wcols[:, 3] = beta
    wcols[0:NC, 4] = bout
    rrows = np.zeros((2, TROW), ml_dtypes.bfloat16)
    rr_raw = rrows.view(np.uint16)
    neg_bits = int(np.float32(NEG).view(np.uint32))
    rr_raw[0, D] = neg_bits & 0xFFFF
    rr_raw[0, D + 1] = neg_bits >> 16
    wmat = np.ascontiguousarray(np.concatenate(
        [W1.astype(ml_dtypes.bfloat16), W2.astype(ml_dtypes.bfloat16)],
        axis=1))
    in_maps = []
    for c in range(NCORES):
        in_maps.append({
            "g1s": np.ascontiguousarray(g1_all[c].reshape(P, NBT1 * D)),
            "es1": np.ascontiguousarray(e1_all[c]),
            "ed1": np.ascontiguousarray(ed1_all[c]),
            "gidx": np.ascontiguousarray(gx_all[c]),
            "nmask": np.ascontiguousarray(nmask_all[c]),
            "wmat": wmat,
            "wcols": wcols,
            "woutb": np.ascontiguousarray(Wout.astype(ml_dtypes.bfloat16)),
            "rrows": rr_raw.view(ml_dtypes.bfloat16),
        })
    meta = dict(NB1=NB1, NB2=NB2, tpos=tpos)
    return in_maps, meta


_CACHE = {}


def kernel(x, edge_index, W1, a_src1, a_dst1, b1, W2, a_src2, a_dst2, b2,
           gamma, beta, Wout, bout, cfg=None):
    cfg = cfg or CFG_FULL
    x = np.asarray(x, np.float32)
    edge_index = np.asarray(edge_index)
    args = [np.asarray(a, np.float32) for a in
            (W1, a_src1, a_dst1, W2, a_src2, a_dst2, gamma, beta, Wout, bout)]
    (W1, a_src1, a_dst1, W2, a_src2, a_dst2, gamma, beta, Wout, bout) = args

    in_maps, meta = prepare(cfg, x, edge_index, W1, a_src1, a_dst1,
                            W2, a_src2, a_dst2, gamma, beta, Wout, bout)
    key = (cfg["N"], tuple(meta["NB1"]), tuple(meta["NB2"]))
    if key not in _CACHE:
        _CACHE[key] = build_program(cfg, meta["NB1"], meta["NB2"])
    nc = _CACHE[key]
    res = run_bass_kernel_spmd(nc, in_maps, list(range(cfg["NCORES"])))

    N, NC, NCORES, NPC = cfg["N"], cfg["NC"], cfg["NCORES"], cfg["TPC"] * P
    full = np.zeros((NCORES * NPC, NC), np.float32)
    for c in range(NCORES):
        full[c * NPC:(c + 1) * NPC] = res.results[c]["out"].T
    return np.ascontiguousarray(full[meta["tpos"]])
